# revision 24
# baseline (speedup 1.0000x reference)
"""Trainium2 Bass kernel for nn_STTM_Single (cross-attention + conv1x1 tail).

Reference computation (per batch b, row h; positions w/x along width W=320):
    q = wq @ left[:, w]   k = wk @ right[:, x]   v = wv @ right[:, x]
    dots[w, x] = (q[:, w] . k[:, x]) * 64**-0.5
    attn = softmax_x(dots)
    out[c, w] = sum_x attn[w, x] v[c, x]
    y = BN(w1 @ concat(left, out)) -> LeakyReLU(0.2) -> w2 @ y

Algebraic folds done on the host (fp64) so the device kernel is lean:
    dots = left^T (0.125 * wq^T wk) right        -> one fused matrix G
    w1 @ concat(left, out) = w1a@left + (w1b wv) @ right-weighted-attn
        with U = s*w1b @ wv, so the 512-wide v/attn-out never materializes
    BN (eval mode) folded into w1 row scale s and a bias vector.

Per-core device pipeline (24 (b,h) rows per core, 8 cores over H=96),
software-pipelined across rows so PE never waits on the softmax DVE chain:
  iteration r emits:  DMA-in(pair) | KG,VUT(r+1) | z(r-1)+DMA-out | dots(r)
                      | y-w1a(r) | attn-transpose(r) | y-attn(r)
All matmul operands are fp16 (full PE rate, fp32 PSUM accumulation) —
fp8 was measured on HW at only 2x (DoubleRow), which the softmax
accuracy budget cannot absorb — softmax statistics and the LeakyReLU
epilogue run in fp32.

Datapath details:
 - Inputs are host-packed per row-PAIR to [128, 2, 8, W] fp16 and fetched
   with four half-row DMAs (>=2.5 KiB/partition lines) on the qSP DGE
   ring; weights and the fp16 output (host-upcast) ride the qAct ring so
   the two streams overlap at startup.
 - The 64-wide W-tail chunks of VUT for the two pair rows are packed into
   one 128-partition matmul group (device-side gather of the x-tails);
   the odd row keeps partition offset 64 all the way through the attn
   transpose so the packed layout feeds y's x-tail contraction directly.
 - A post-pass (_dedup_waits) drops semaphore waits subsumed by earlier
   waits on the same in-order engine queue, and _cap_waits demotes
   multi-wait instructions to single-wait + EventSemaphore for walrus.
"""

import numpy as np

import concourse.bass as bass
import concourse.bass_isa as bass_isa
import concourse.mybir as mybir
import concourse.tile as tile
from concourse import bass_utils

B, C, H, W = 2, 512, 96, 320
OUT = 256
N_CORES = 8
H_PER = H // N_CORES  # 12
ROWS = B * H_PER      # 24 (b,h) rows per core
SCALE = 64 ** -0.5
BN_EPS = 1e-5

F32 = mybir.dt.float32
F16 = mybir.dt.float16
BF16 = mybir.dt.bfloat16

# width chunking for the 128-partition dim: 320 = 128 + 128 + 64
W_CH = [(0, 128), (128, 128), (256, 64)]


def _cap_waits(nc: bass.Bass, max_waits: int = 1) -> int:
    """Walrus codegen allows only one sync-wait slot on most instruction
    encodings (DMA descriptors, S3D3 matmul, ...). Tile occasionally attaches
    2-3 waits to one instruction; demote the excess onto standalone
    EventSemaphore instructions (engine-sequencer waits, unlimited) placed
    just before the instruction — and before its paired LDWEIGHTS so the
    weight load stays adjacent to its matmul."""
    n_fixed = 0
    for f in nc.m.functions:
        for blk in f.blocks:
            insts = list(blk.instructions)
            out = []
            for inst in insts:
                kind = inst.__class__.__name__
                si = getattr(inst, "sync_info", None)
                if (
                    kind != "InstEventSemaphore"
                    and si
                    and si.on_wait
                    and len(si.on_wait) > max_waits
                ):
                    waits = list(si.on_wait)
                    excess, keep = waits[:-max_waits], waits[-max_waits:]
                    evs = []
                    for k, w in enumerate(excess):
                        ev = mybir.InstEventSemaphore(
                            name=f"{inst.name}-evw{k}", engine=inst.engine
                        )
                        ev.sync_info = mybir.SyncInfo(on_wait=[w], on_update=[])
                        nc.register_instruction(ev)
                        evs.append(ev)
                    si.on_wait = keep
                    # hop back over an adjacent same-engine LDWEIGHTS pair
                    ip = len(out)
                    while (
                        ip > 0
                        and out[ip - 1].__class__.__name__ == "InstLdweights"
                        and out[ip - 1].engine == inst.engine
                    ):
                        ip -= 1
                    out[ip:ip] = evs
                    n_fixed += 1
                out.append(inst)
            if n_fixed:
                blk.instructions = out
    return n_fixed


def _dedup_waits(nc: bass.Bass) -> int:
    """Drop semaphore waits already implied by earlier waits on the same
    (in-order) engine queue: a `sem >= k` wait is redundant if an earlier
    instruction on this queue already waited `sem >= k' with k' >= k`.
    Only applies to monotone counting semaphores (never decremented)."""
    # find sems that are ever decremented / subtracted (barrier sems)
    unsafe: set[int] = set()
    for f in nc.m.functions:
        for blk in f.blocks:
            for inst in blk.instructions:
                si = getattr(inst, "sync_info", None)
                if not si:
                    continue
                for u in si.on_update or []:
                    if u.update_mode not in ("sem-inc", "sem-add-imm"):
                        unsafe.add(u.id)
    n_drop = 0
    for f in nc.m.functions:
        for blk in f.blocks:
            seen: dict[tuple, int] = {}  # (engine, sem id) -> max waited value
            for inst in blk.instructions:
                si = getattr(inst, "sync_info", None)
                if not si or not si.on_wait:
                    continue
                eng = inst.engine
                keep = []
                for w in si.on_wait:
                    if (
                        w.wait_mode == "sem-ge-imm"
                        and w.id not in unsafe
                        and w.wait_reg is None
                    ):
                        key = (eng, w.id)
                        prev = seen.get(key)
                        if prev is not None and prev >= w.wait_value:
                            n_drop += 1
                            continue
                        seen[key] = max(prev or 0, w.wait_value)
                    keep.append(w)
                if len(keep) != len(si.on_wait):
                    si.on_wait = keep
    return n_drop


def build_nc(rows: int = ROWS) -> bass.Bass:
    nc = bass.Bass()
    xin = nc.declare_dram_parameter("xin", [rows, 128, 8, W], F16, isOutput=False)
    gT = nc.declare_dram_parameter("gT", [C, C], F16, isOutput=False)
    uT = nc.declare_dram_parameter("uT", [C, OUT], F16, isOutput=False)
    w1aT = nc.declare_dram_parameter("w1aT", [C, OUT], F16, isOutput=False)
    w2T = nc.declare_dram_parameter("w2T", [OUT, OUT], F16, isOutput=False)
    bnb = nc.declare_dram_parameter("bnb", [OUT], F32, isOutput=False)
    out = nc.declare_dram_parameter("out", [rows, 128, 2, W], F16, isOutput=True)

    Exp = mybir.ActivationFunctionType.Exp

    with tile.TileContext(nc) as tc:
        with (
            tc.tile_pool(name="wpool", bufs=1) as wp,
            tc.tile_pool(name="io", bufs=4) as io,
            tc.tile_pool(name="work", bufs=2) as wk,
            tc.tile_pool(name="psum", bufs=1, space="PSUM") as pp,
        ):
            st: dict[int, dict] = {}  # per-row live tiles

            def dma_in(p):
                # row-PAIR tile [128, 2 rows, 8 chunks, W]; issued as four
                # half-row DMAs (xr before xl) so the first KG of each row can
                # start as soon as its xr lands.  The very first xr is further
                # split per chunk to cut the cold-start latency.
                x_t = io.tile([128, 2, 8, W], F16, tag="x", name="x_t")
                for k in range(2):
                    rv = xin[2 * p + k]
                    if p == 0 and k == 0:
                        for j in range(4):
                            nc.sync.dma_start(
                                out=x_t[:, k, 4 + j, :], in_=rv[:, 4 + j, :]
                            )
                    else:
                        nc.sync.dma_start(out=x_t[:, k, 4:8, :], in_=rv[:, 4:8, :])
                for k in range(2):
                    rv = xin[2 * p + k]
                    nc.sync.dma_start(out=x_t[:, k, 0:4, :], in_=rv[:, 0:4, :])
                st[2 * p] = {"x": x_t, "k": 0}
                st[2 * p + 1] = {"x": x_t, "k": 1}

            # ---- prefetch two row-pairs on the qSP DGE ring ----
            dma_in(0)
            dma_in(1)

            # ---- persistent weights on the qAct DGE ring (parallel with
            # the input prefetch); gT split per chunk so KG(0) starts early ----
            gt_sb = wp.tile([128, 4, C], F16, name="gt_sb")
            gt_v = gT.rearrange("(j p) m -> p j m", p=128)
            for j in range(4):
                nc.scalar.dma_start(out=gt_sb[:, j, :], in_=gt_v[:, j, :])
            ut_sb = wp.tile([128, 4, OUT], F16, name="ut_sb")
            nc.scalar.dma_start(out=ut_sb, in_=uT.rearrange("(j p) m -> p j m", p=128))
            w1a_sb = wp.tile([128, 4, OUT], F16, name="w1a_sb")
            nc.scalar.dma_start(
                out=w1a_sb, in_=w1aT.rearrange("(j p) m -> p j m", p=128)
            )
            w2_sb = wp.tile([128, 2, OUT], F16, name="w2_sb")
            nc.scalar.dma_start(out=w2_sb, in_=w2T.rearrange("(j p) m -> p j m", p=128))
            bias_sb = wp.tile([128, 2], F32, name="bias_sb")
            nc.scalar.dma_start(out=bias_sb, in_=bnb.rearrange("(j p) -> p j", p=128))
            # all-ones stationary: one PE pass both sums exp over the
            # partition (x) axis and replicates the result to all partitions
            ones_sb = wp.tile([128, 128], BF16, name="ones_sb")
            nc.gpsimd.memset(ones_sb, 1.0)
            # preload the Exp activation table while DMAs are in flight so the
            # first real softmax doesn't eat the 1.3us lazy ACT_TABLE_LOAD
            warm = wp.tile([1, 2], F32, name="warm")
            nc.gpsimd.memset(warm[:, 0:1], 0.0)
            nc.scalar.activation(warm[:, 1:2], warm[:, 0:1], Exp)

            def kg_proj(r):
                s = st[r]
                k = s["k"]
                xr = s["x"][:, k, 4:8, :]
                # KG = G @ Xr : [c1, x].  All projection groups share one
                # 3-deep PSUM ring ("proj"): each group reuses the bank from
                # 3 allocations back, whose drain copy is always finished, so
                # no group ever stalls on a copy WAR.
                kg_sb = wk.tile([128, 4, W], F16, tag="kg", name="kg_sb")
                for i in range(4):  # c1 chunk
                    pkg = pp.tile([128, W], F32, tag="proj", bufs=2, name="pkg")
                    for j in range(4):  # c2 chunk (contraction)
                        nc.tensor.matmul(
                            pkg,
                            gt_sb[:, j, 128 * i : 128 * (i + 1)],
                            xr[:, j, :],
                            start=(j == 0),
                            stop=(j == 3),
                        )
                    nc.scalar.copy(kg_sb[:, i, :], pkg)
                s["kg"] = kg_sb

            def vut_proj(r):
                s = st[r]
                k = s["k"]
                xr = s["x"][:, k, 4:8, :]
                # VUT[x, u] = Xr^T @ U^T : x on partitions; both 128-chunks
                # accumulate in one bank (two column regions), one drain copy
                vut_sb = wk.tile([128, 2, OUT], BF16, tag="vut", name="vut_sb")
                pvu = pp.tile([128, 2, OUT], F32, tag="proj", bufs=2, name="pvu")
                for xc in range(2):
                    x0 = 128 * xc
                    for j in range(4):  # c2 chunk (contraction)
                        nc.tensor.matmul(
                            pvu[:, xc, :],
                            xr[:, j, x0 : x0 + 128],
                            ut_sb[:, j, :],
                            start=(j == 0),
                            stop=(j == 3),
                        )
                nc.vector.tensor_copy(vut_sb, pvu)
                s["vut"] = vut_sb
                if k == 1:
                    # the 64-wide x-tail of BOTH pair rows in one group.
                    # Walrus needs a single-free-dim stationary AP, so gather
                    # the two tails into one contiguous 128-wide block first:
                    # out partitions 0-63 = even row, 64-127 = odd row.
                    xt = wk.tile([128, 4, 128], F16, tag="xt", name="xt")
                    nc.vector.tensor_copy(
                        xt[:, :, 0:64], s["x"][:, 0, 4:8, 256:320]
                    )
                    nc.vector.tensor_copy(
                        xt[:, :, 64:128], s["x"][:, 1, 4:8, 256:320]
                    )
                    vut_tail = wk.tile([128, OUT], BF16, tag="vutt", name="vut_tail")
                    pvt = pp.tile([128, OUT], F32, tag="proj", bufs=2, name="pvt")
                    for j in range(4):
                        nc.tensor.matmul(
                            pvt,
                            xt[:, j, :],
                            ut_sb[:, j, :],
                            start=(j == 0),
                            stop=(j == 3),
                        )
                    nc.vector.tensor_copy(vut_tail, pvt)
                    s["vutt"] = vut_tail
                    st[r - 1]["vutt"] = vut_tail

            def dots_exp(r):
                # dotsT[x, w] = KG^T @ Xl computed directly TRANSPOSED: x on
                # partitions (3 chunks; the 64-wide tail at partition offset
                # 64*k so pair rows share the packed vut_tail layout).  exp to
                # bf16 (fp32-like range, so no running-max pass needed).
                # This removes all PE transposes and their psum drains.
                s = st[r]
                off = 64 * s["k"]
                xl = s["x"][:, s["k"], 0:4, :]
                eT = wk.tile([128, 3, W], BF16, tag="eT", name="eT")
                for xc, (x0, xs) in enumerate(W_CH):
                    o = off if xs == 64 else 0
                    pd = pp.tile([128, W], F32, tag="acc", bufs=3, name="pd")
                    for i in range(4):  # c chunk (contraction)
                        nc.tensor.matmul(
                            pd[o : o + xs, :],
                            s["kg"][:, i, x0 : x0 + xs],
                            xl[:, i, :],
                            start=(i == 0),
                            stop=(i == 3),
                        )
                    nc.scalar.activation(eT[o : o + xs, xc, :], pd[o : o + xs, :], Exp)
                s["eT"] = eT

            def sums_norm(r):
                # softmax denominators: ones^T @ eT sums exp over x AND
                # replicates the per-w total to every partition, so the
                # reciprocal lands full-width and the bf16 weights get their
                # 1/sum scale with plain elementwise multiplies.
                s = st[r]
                off = 64 * s["k"]
                eT = s["eT"]
                psum_s = pp.tile([128, W], F32, tag="sum", bufs=1, name="psum_s")
                for xc, (x0, xs) in enumerate(W_CH):
                    o = off if xs == 64 else 0
                    nc.tensor.matmul(
                        psum_s,
                        ones_sb[o : o + xs, :],
                        eT[o : o + xs, xc, :],
                        start=(xc == 0),
                        stop=(xc == 2),
                    )
                rb_sb = wk.tile([128, W], F32, tag="rb", name="rb_sb")
                nc.vector.reciprocal(rb_sb, psum_s)
                at_sb = wk.tile([128, 3, W], BF16, tag="at", name="at_sb")
                for xc, (x0, xs) in enumerate(W_CH):
                    o = off if xs == 64 else 0
                    nc.vector.tensor_tensor(
                        at_sb[o : o + xs, xc, :],
                        eT[o : o + xs, xc, :],
                        rb_sb[o : o + xs, :],
                        op=mybir.AluOpType.mult,
                    )
                s["at"] = at_sb

            def y_w1a(r):
                # open the two y PSUM groups with the w1a part (no attn dep)
                s = st[r]
                xl = s["x"][:, s["k"], 0:4, :]
                pys = []
                for uc in range(2):
                    py = pp.tile([128, W], F32, tag="acc", bufs=3, name="py")
                    for i in range(4):  # c1 contraction (w1a part)
                        nc.tensor.matmul(
                            py,
                            w1a_sb[:, i, 128 * uc : 128 * (uc + 1)],
                            xl[:, i, :],
                            start=(i == 0),
                            stop=False,
                        )
                    pys.append(py)
                s["py"] = pys

            def y_attn(r):
                # finish y = w1a@Xl + VUT^T @ AT ; LeakyReLU(y + bias)
                s = st[r]
                off = 64 * s["k"]
                y_sb = wk.tile([128, 2, W], F16, tag="y", name="y_sb")
                u_sb = wk.tile([128, W], F32, tag="lr", name="u_sb")
                # xc-major so the first matmuls need only the first at-mult
                for xc in range(2):  # full 128-chunks of the x contraction
                    for uc in range(2):
                        nc.tensor.matmul(
                            s["py"][uc],
                            s["vut"][:, xc, 128 * uc : 128 * (uc + 1)],
                            s["at"][:, xc, :],
                            start=False,
                            stop=False,
                        )
                for uc in range(2):
                    py = s["py"][uc]
                    nc.tensor.matmul(  # 64-wide x-tail at partition offset
                        py,
                        s["vutt"][off : off + 64, 128 * uc : 128 * (uc + 1)],
                        s["at"][off : off + 64, 2, :],
                        start=False,
                        stop=True,
                    )
                    # LeakyReLU(t) = max(t, 0.2*t) with t = py + bias
                    bias_ap = bias_sb[:, uc : uc + 1]
                    nc.vector.tensor_scalar(
                        out=u_sb,
                        in0=py,
                        scalar1=bias_ap,
                        scalar2=0.2,
                        op0=mybir.AluOpType.add,
                        op1=mybir.AluOpType.mult,
                    )
                    nc.vector.scalar_tensor_tensor(
                        out=y_sb[:, uc, :],
                        in0=py,
                        scalar=bias_ap,
                        in1=u_sb,
                        op0=mybir.AluOpType.add,
                        op1=mybir.AluOpType.max,
                    )
                s["y"] = y_sb

            def z_out(r):
                # z = w2 @ y -> DRAM (fp16, host upcasts)
                s = st[r]
                z_sb = wk.tile([128, 2, W], F16, tag="z", name="z_sb")
                for oc in range(2):
                    pz = pp.tile([128, W], F32, tag="misc", bufs=2, name="pz")
                    for uc in range(2):
                        nc.tensor.matmul(
                            pz,
                            w2_sb[:, uc, 128 * oc : 128 * (oc + 1)],
                            s["y"][:, uc, :],
                            start=(uc == 0),
                            stop=(uc == 1),
                        )
                    nc.vector.tensor_copy(z_sb[:, oc, :], pz)
                nc.scalar.dma_start(out=out[r], in_=z_sb)
                # release per-row tiles
                del st[r]

            # ---- software-pipelined main loop ----
            # Per-iteration PE order: KG(r+1) first covers the previous row's
            # DVE epilogue before dots(r) reuses its PSUM ring slot; z(r-1)
            # sits between dots and the ones-sums so the last exp chunk is
            # ready; y_w1a+VUT (~5.1k cols) cover the 1/sum+scale DVE chain
            # before y_attn(r) consumes the normalized weights.
            kg_proj(0)
            vut_proj(0)
            for r in range(rows):
                if r % 2 == 0 and r // 2 + 2 < rows // 2:
                    dma_in(r // 2 + 2)
                if r + 1 < rows:
                    kg_proj(r + 1)
                dots_exp(r)
                if r >= 1:
                    z_out(r - 1)
                sums_norm(r)
                y_w1a(r)
                if r + 1 < rows:
                    vut_proj(r + 1)
                y_attn(r)
            z_out(rows - 1)
    _dedup_waits(nc)
    _cap_waits(nc)
    return nc


def fold_weights(wq, wk_, wv, w1, bn_gamma, bn_beta, bn_mean, bn_var, w2):
    """Host-side fp64 weight folding; returns the small device tensors."""
    f8 = np.float64
    s = bn_gamma.astype(f8) / np.sqrt(bn_var.astype(f8) + BN_EPS)
    w1s = w1.astype(f8) * s[:, None]
    w1a = w1s[:, :C]           # applies to left_feat
    w1b = w1s[:, C:]           # applies to the attention output
    U = w1b @ wv.astype(f8)    # [OUT, C]
    gTm = SCALE * (wk_.astype(f8).T @ wq.astype(f8))  # [c2, c1]
    bias = bn_beta.astype(f8) - bn_mean.astype(f8) * s
    return {
        "gT": np.ascontiguousarray(gTm, np.float16),
        "uT": np.ascontiguousarray(U.T, np.float16),
        "w1aT": np.ascontiguousarray(w1a.T, np.float16),
        "w2T": np.ascontiguousarray(w2.astype(f8).T, np.float16),
        "bnb": np.ascontiguousarray(bias, np.float32),
    }


def make_in_maps(inputs):
    left = np.asarray(inputs["left_feat"], np.float16)
    right = np.asarray(inputs["right_feat"], np.float16)
    common = fold_weights(
        np.asarray(inputs["wq"]),
        np.asarray(inputs["wk"]),
        np.asarray(inputs["wv"]),
        np.asarray(inputs["w1"]),
        np.asarray(inputs["bn_gamma"]),
        np.asarray(inputs["bn_beta"]),
        np.asarray(inputs["bn_mean"]),
        np.asarray(inputs["bn_var"]),
        np.asarray(inputs["w2"]),
    )
    in_maps = []
    for core in range(N_CORES):
        hs = slice(core * H_PER, (core + 1) * H_PER)
        # [ROWS, C, W] with row = (b, h)
        lf = left[:, :, hs, :].transpose(0, 2, 1, 3).reshape(ROWS, C, W)
        rf = right[:, :, hs, :].transpose(0, 2, 1, 3).reshape(ROWS, C, W)
        # pack to [ROWS, 128, 8, W]: chunks 0-3 = lf, 4-7 = rf; c = j*128 + p
        xin = np.concatenate(
            [
                lf.reshape(ROWS, 4, 128, W).transpose(0, 2, 1, 3),
                rf.reshape(ROWS, 4, 128, W).transpose(0, 2, 1, 3),
            ],
            axis=2,
        )
        in_maps.append({"xin": np.ascontiguousarray(xin), **common})
    return in_maps


def assemble_out(results):
    out = np.empty((B, OUT, H, W), np.float32)
    for core in range(N_CORES):
        # device layout [ROWS, 128, 2, W], channel o = oc*128 + p
        o = np.asarray(results[core]["out"]).astype(np.float32)
        o = o.transpose(0, 2, 1, 3).reshape(B, H_PER, OUT, W)
        out[:, :, core * H_PER : (core + 1) * H_PER, :] = o.transpose(0, 2, 1, 3)
    return out


_NC_CACHE: dict[int, bass.Bass] = {}


def get_nc(rows: int = ROWS) -> bass.Bass:
    if rows not in _NC_CACHE:
        _NC_CACHE[rows] = build_nc(rows)
    return _NC_CACHE[rows]


def run_sharded(inputs, **run_kwargs) -> bass_utils.BassKernelResults:
    """Run the SPMD kernel on all 8 cores; extra kwargs go to the runner
    (e.g. trace=True, trace_cores=[0] for NTFF profiling in test.py)."""
    in_maps = make_in_maps(inputs)
    nc = get_nc()
    return bass_utils.run_bass_kernel_spmd(
        nc, in_maps, core_ids=list(range(N_CORES)), **run_kwargs
    )


def kernel(**inputs) -> np.ndarray:
    return assemble_out(run_sharded(inputs).results)



# revision 26
# speedup vs baseline: 1.1455x; 1.1455x over previous
"""Trainium2 Bass kernel for nn_STTM_Single (cross-attention + conv1x1 tail).

Reference computation (per batch b, row h; positions w/x along width W=320):
    q = wq @ left[:, w]   k = wk @ right[:, x]   v = wv @ right[:, x]
    dots[w, x] = (q[:, w] . k[:, x]) * 64**-0.5
    attn = softmax_x(dots)
    out[c, w] = sum_x attn[w, x] v[c, x]
    y = BN(w1 @ concat(left, out)) -> LeakyReLU(0.2) -> w2 @ y

Algebraic folds done on the host (fp64) so the device kernel is lean:
    dots = left^T (0.125 * wq^T wk) right        -> one fused matrix G
    w1 @ concat(left, out) = w1a@left + (w1b wv) @ right-weighted-attn
        with U = s*w1b @ wv, so the 512-wide v/attn-out never materializes
    BN (eval mode) folded into w1 row scale s and a bias vector.

Per-core device pipeline (24 (b,h) rows per core, 8 cores over H=96),
software-pipelined across rows so PE never waits on the softmax DVE chain:
  iteration r emits:  DMA-in(pair) | KG,VUT(r+1) | z(r-1)+DMA-out | dots(r)
                      | y-w1a(r) | attn-transpose(r) | y-attn(r)
All matmul operands are fp16 (full PE rate, fp32 PSUM accumulation) —
fp8 was measured on HW at only 2x (DoubleRow), which the softmax
accuracy budget cannot absorb — softmax statistics and the LeakyReLU
epilogue run in fp32.

Datapath details:
 - Inputs are host-packed per row-PAIR to [128, 2, 8, W] fp16 and fetched
   with four half-row DMAs (>=2.5 KiB/partition lines) on the qSP DGE
   ring; weights and the fp16 output (host-upcast) ride the qAct ring so
   the two streams overlap at startup.
 - The 64-wide W-tail chunks of VUT for the two pair rows are packed into
   one 128-partition matmul group (device-side gather of the x-tails);
   the odd row keeps partition offset 64 all the way through the attn
   transpose so the packed layout feeds y's x-tail contraction directly.
 - A post-pass (_dedup_waits) drops semaphore waits subsumed by earlier
   waits on the same in-order engine queue, and _cap_waits demotes
   multi-wait instructions to single-wait + EventSemaphore for walrus.
"""

import numpy as np

import concourse.bass as bass
import concourse.bass_isa as bass_isa
import concourse.mybir as mybir
import concourse.tile as tile
from concourse import bass_utils

B, C, H, W = 2, 512, 96, 320
OUT = 256
N_CORES = 8
H_PER = H // N_CORES  # 12
ROWS = B * H_PER      # 24 (b,h) rows per core
SCALE = 64 ** -0.5
BN_EPS = 1e-5

F32 = mybir.dt.float32
F16 = mybir.dt.float16
BF16 = mybir.dt.bfloat16

# width chunking for the 128-partition dim: 320 = 128 + 128 + 64
W_CH = [(0, 128), (128, 128), (256, 64)]


def _cap_waits(nc: bass.Bass, max_waits: int = 1) -> int:
    """Walrus codegen allows only one sync-wait slot on most instruction
    encodings (DMA descriptors, S3D3 matmul, ...). Tile occasionally attaches
    2-3 waits to one instruction; demote the excess onto standalone
    EventSemaphore instructions (engine-sequencer waits, unlimited) placed
    just before the instruction — and before its paired LDWEIGHTS so the
    weight load stays adjacent to its matmul."""
    n_fixed = 0
    for f in nc.m.functions:
        for blk in f.blocks:
            insts = list(blk.instructions)
            out = []
            for inst in insts:
                kind = inst.__class__.__name__
                si = getattr(inst, "sync_info", None)
                if (
                    kind != "InstEventSemaphore"
                    and si
                    and si.on_wait
                    and len(si.on_wait) > max_waits
                ):
                    waits = list(si.on_wait)
                    excess, keep = waits[:-max_waits], waits[-max_waits:]
                    evs = []
                    for k, w in enumerate(excess):
                        ev = mybir.InstEventSemaphore(
                            name=f"{inst.name}-evw{k}", engine=inst.engine
                        )
                        ev.sync_info = mybir.SyncInfo(on_wait=[w], on_update=[])
                        nc.register_instruction(ev)
                        evs.append(ev)
                    si.on_wait = keep
                    # hop back over an adjacent same-engine LDWEIGHTS pair
                    ip = len(out)
                    while (
                        ip > 0
                        and out[ip - 1].__class__.__name__ == "InstLdweights"
                        and out[ip - 1].engine == inst.engine
                    ):
                        ip -= 1
                    out[ip:ip] = evs
                    n_fixed += 1
                out.append(inst)
            if n_fixed:
                blk.instructions = out
    return n_fixed


def _dedup_waits(nc: bass.Bass) -> int:
    """Drop semaphore waits already implied by earlier waits on the same
    (in-order) engine queue: a `sem >= k` wait is redundant if an earlier
    instruction on this queue already waited `sem >= k' with k' >= k`.
    Only applies to monotone counting semaphores (never decremented)."""
    # find sems that are ever decremented / subtracted (barrier sems)
    unsafe: set[int] = set()
    for f in nc.m.functions:
        for blk in f.blocks:
            for inst in blk.instructions:
                si = getattr(inst, "sync_info", None)
                if not si:
                    continue
                for u in si.on_update or []:
                    if u.update_mode not in ("sem-inc", "sem-add-imm"):
                        unsafe.add(u.id)
    n_drop = 0
    for f in nc.m.functions:
        for blk in f.blocks:
            seen: dict[tuple, int] = {}  # (engine, sem id) -> max waited value
            for inst in blk.instructions:
                si = getattr(inst, "sync_info", None)
                if not si or not si.on_wait:
                    continue
                eng = inst.engine
                keep = []
                for w in si.on_wait:
                    if (
                        w.wait_mode == "sem-ge-imm"
                        and w.id not in unsafe
                        and w.wait_reg is None
                    ):
                        key = (eng, w.id)
                        prev = seen.get(key)
                        if prev is not None and prev >= w.wait_value:
                            n_drop += 1
                            continue
                        seen[key] = max(prev or 0, w.wait_value)
                    keep.append(w)
                if len(keep) != len(si.on_wait):
                    si.on_wait = keep
    return n_drop


def build_nc(rows: int = ROWS) -> bass.Bass:
    nc = bass.Bass()
    xin = nc.declare_dram_parameter("xin", [rows, 128, 8, W], F16, isOutput=False)
    gT = nc.declare_dram_parameter("gT", [C, C], F16, isOutput=False)
    uT = nc.declare_dram_parameter("uT", [C, OUT], F16, isOutput=False)
    w1aT = nc.declare_dram_parameter("w1aT", [C, OUT], F16, isOutput=False)
    w2T = nc.declare_dram_parameter("w2T", [OUT, OUT], F16, isOutput=False)
    bnb = nc.declare_dram_parameter("bnb", [OUT], F32, isOutput=False)
    out = nc.declare_dram_parameter("out", [rows, 128, 2, W], F16, isOutput=True)

    Exp = mybir.ActivationFunctionType.Exp

    with tile.TileContext(nc) as tc:
        with (
            tc.tile_pool(name="wpool", bufs=1) as wp,
            tc.tile_pool(name="io", bufs=4) as io,
            tc.tile_pool(name="work", bufs=2) as wk,
            tc.tile_pool(name="psum", bufs=1, space="PSUM") as pp,
        ):
            st: dict[int, dict] = {}  # per-row live tiles

            def dma_in(p):
                # row-PAIR tile [128, 2 rows, 8 chunks, W]; issued as four
                # half-row DMAs (xr before xl) so the first KG of each row can
                # start as soon as its xr lands.  The very first xr is further
                # split per chunk to cut the cold-start latency.
                x_t = io.tile([128, 2, 8, W], F16, tag="x", name="x_t")
                for k in range(2):
                    rv = xin[2 * p + k]
                    if p == 0 and k == 0:
                        for j in range(4):
                            nc.sync.dma_start(
                                out=x_t[:, k, 4 + j, :], in_=rv[:, 4 + j, :]
                            )
                    else:
                        nc.sync.dma_start(out=x_t[:, k, 4:8, :], in_=rv[:, 4:8, :])
                for k in range(2):
                    rv = xin[2 * p + k]
                    nc.sync.dma_start(out=x_t[:, k, 0:4, :], in_=rv[:, 0:4, :])
                st[2 * p] = {"x": x_t, "k": 0}
                st[2 * p + 1] = {"x": x_t, "k": 1}

            # ---- prefetch two row-pairs on the qSP DGE ring ----
            dma_in(0)
            dma_in(1)

            # ---- persistent weights on the qAct DGE ring (parallel with
            # the input prefetch); gT split per chunk so KG(0) starts early ----
            gt_sb = wp.tile([128, 4, C], F16, name="gt_sb")
            gt_v = gT.rearrange("(j p) m -> p j m", p=128)
            for j in range(4):
                nc.scalar.dma_start(out=gt_sb[:, j, :], in_=gt_v[:, j, :])
            ut_sb = wp.tile([128, 4, OUT], F16, name="ut_sb")
            nc.scalar.dma_start(out=ut_sb, in_=uT.rearrange("(j p) m -> p j m", p=128))
            w1a_sb = wp.tile([128, 4, OUT], F16, name="w1a_sb")
            nc.scalar.dma_start(
                out=w1a_sb, in_=w1aT.rearrange("(j p) m -> p j m", p=128)
            )
            w2_sb = wp.tile([128, 2, OUT], F16, name="w2_sb")
            nc.scalar.dma_start(out=w2_sb, in_=w2T.rearrange("(j p) m -> p j m", p=128))
            bias_sb = wp.tile([128, 2], F32, name="bias_sb")
            nc.scalar.dma_start(out=bias_sb, in_=bnb.rearrange("(j p) -> p j", p=128))
            # all-ones stationary: one PE pass both sums exp over the
            # partition (x) axis and replicates the result to all partitions
            ones_sb = wp.tile([128, 128], BF16, name="ones_sb")
            nc.gpsimd.memset(ones_sb, 1.0)
            # preload the activation table while DMAs are in flight so the
            # first real softmax doesn't eat the 1.3us lazy ACT_TABLE_LOAD.
            # Warming Ln pins the natural_log_exp set (the only one with
            # Ln+Exp+Copy together).
            warm = wp.tile([1, 2], F32, name="warm")
            nc.gpsimd.memset(warm[:, 0:1], 1.0)
            nc.scalar.activation(
                warm[:, 1:2], warm[:, 0:1], mybir.ActivationFunctionType.Ln
            )
            nc.scalar.activation(warm[:, 1:2], warm[:, 0:1], Exp)

            def kg_proj(r):
                s = st[r]
                k = s["k"]
                xr = s["x"][:, k, 4:8, :]
                # KG = G @ Xr : [c1, x].  All projection groups share one
                # 3-deep PSUM ring ("proj"): each group reuses the bank from
                # 3 allocations back, whose drain copy is always finished, so
                # no group ever stalls on a copy WAR.
                kg_sb = wk.tile([128, 4, W], F16, tag="kg", name="kg_sb")
                for i in range(4):  # c1 chunk
                    pkg = pp.tile([128, W], F32, tag="proj", bufs=2, name="pkg")
                    for j in range(4):  # c2 chunk (contraction)
                        nc.tensor.matmul(
                            pkg,
                            gt_sb[:, j, 128 * i : 128 * (i + 1)],
                            xr[:, j, :],
                            start=(j == 0),
                            stop=(j == 3),
                        )
                    nc.scalar.copy(kg_sb[:, i, :], pkg)
                s["kg"] = kg_sb

            def vut_proj(r):
                s = st[r]
                k = s["k"]
                xr = s["x"][:, k, 4:8, :]
                # VUT[x, u] = Xr^T @ U^T : x on partitions; both 128-chunks
                # accumulate in one bank (two column regions), one drain copy
                vut_sb = wk.tile([128, 2, OUT], BF16, tag="vut", name="vut_sb")
                pvu = pp.tile([128, 2, OUT], F32, tag="proj", bufs=2, name="pvu")
                for xc in range(2):
                    x0 = 128 * xc
                    for j in range(4):  # c2 chunk (contraction)
                        nc.tensor.matmul(
                            pvu[:, xc, :],
                            xr[:, j, x0 : x0 + 128],
                            ut_sb[:, j, :],
                            start=(j == 0),
                            stop=(j == 3),
                        )
                nc.vector.tensor_copy(vut_sb, pvu)
                s["vut"] = vut_sb
                if k == 1:
                    # the 64-wide x-tail of BOTH pair rows in one group.
                    # Walrus needs a single-free-dim stationary AP, so gather
                    # the two tails into one contiguous 128-wide block first:
                    # out partitions 0-63 = even row, 64-127 = odd row.
                    xt = wk.tile([128, 4, 128], F16, tag="xt", name="xt")
                    nc.vector.tensor_copy(
                        xt[:, :, 0:64], s["x"][:, 0, 4:8, 256:320]
                    )
                    nc.vector.tensor_copy(
                        xt[:, :, 64:128], s["x"][:, 1, 4:8, 256:320]
                    )
                    vut_tail = wk.tile([128, OUT], BF16, tag="vutt", name="vut_tail")
                    pvt = pp.tile([128, OUT], F32, tag="proj", bufs=2, name="pvt")
                    for j in range(4):
                        nc.tensor.matmul(
                            pvt,
                            xt[:, j, :],
                            ut_sb[:, j, :],
                            start=(j == 0),
                            stop=(j == 3),
                        )
                    nc.vector.tensor_copy(vut_tail, pvt)
                    s["vutt"] = vut_tail
                    st[r - 1]["vutt"] = vut_tail

            def dots_exp(r):
                # dotsT[x, w] = KG^T @ Xl computed directly TRANSPOSED: x on
                # partitions (3 chunks; the 64-wide tail at partition offset
                # 64*k so pair rows share the packed vut_tail layout).  exp to
                # bf16 (fp32-like range, so no running-max pass needed).
                # This removes all PE transposes and their psum drains.
                s = st[r]
                off = 64 * s["k"]
                xl = s["x"][:, s["k"], 0:4, :]
                eT = wk.tile([128, 3, W], BF16, tag="eT", name="eT")
                for xc, (x0, xs) in enumerate(W_CH):
                    o = off if xs == 64 else 0
                    pd = pp.tile([128, W], F32, tag="acc", bufs=3, name="pd")
                    for i in range(4):  # c chunk (contraction)
                        nc.tensor.matmul(
                            pd[o : o + xs, :],
                            s["kg"][:, i, x0 : x0 + xs],
                            xl[:, i, :],
                            start=(i == 0),
                            stop=(i == 3),
                        )
                    nc.scalar.activation(eT[o : o + xs, xc, :], pd[o : o + xs, :], Exp)
                s["eT"] = eT

            def sums_norm(r):
                # softmax denominators: ones^T @ eT sums exp over x AND
                # replicates the per-w total to every partition, so the
                # reciprocal lands full-width and the bf16 weights get their
                # 1/sum scale with plain elementwise multiplies.
                s = st[r]
                off = 64 * s["k"]
                eT = s["eT"]
                psum_s = pp.tile([128, W], F32, tag="sum", bufs=1, name="psum_s")
                for xc, (x0, xs) in enumerate(W_CH):
                    o = off if xs == 64 else 0
                    nc.tensor.matmul(
                        psum_s,
                        ones_sb[o : o + xs, :],
                        eT[o : o + xs, xc, :],
                        start=(xc == 0),
                        stop=(xc == 2),
                    )
                # 1/sums = exp(-ln(sums)) on Act (both funcs share one table
                # set with Copy, so no table switches); DVE reciprocal is an
                # iterative ~6.4 cyc/elem op (2.1us/row) -- far too slow here.
                rb_ln = wk.tile([128, W], F32, tag="rbl", name="rb_ln")
                nc.scalar.activation(
                    rb_ln, psum_s, mybir.ActivationFunctionType.Ln
                )
                rb_sb = wk.tile([128, W], BF16, tag="rb", name="rb_sb")
                nc.scalar.activation(rb_sb, rb_ln, Exp, scale=-1.0)
                at_sb = wk.tile([128, 3, W], BF16, tag="at", name="at_sb")
                for xc, (x0, xs) in enumerate(W_CH):
                    o = off if xs == 64 else 0
                    nc.vector.tensor_tensor(
                        at_sb[o : o + xs, xc, :],
                        eT[o : o + xs, xc, :],
                        rb_sb[o : o + xs, :],
                        op=mybir.AluOpType.mult,
                    )
                s["at"] = at_sb

            def y_w1a(r):
                # open the two y PSUM groups with the w1a part (no attn dep)
                s = st[r]
                xl = s["x"][:, s["k"], 0:4, :]
                pys = []
                for uc in range(2):
                    py = pp.tile([128, W], F32, tag="acc", bufs=3, name="py")
                    for i in range(4):  # c1 contraction (w1a part)
                        nc.tensor.matmul(
                            py,
                            w1a_sb[:, i, 128 * uc : 128 * (uc + 1)],
                            xl[:, i, :],
                            start=(i == 0),
                            stop=False,
                        )
                    pys.append(py)
                s["py"] = pys

            def y_attn(r):
                # finish y = w1a@Xl + VUT^T @ AT ; LeakyReLU(y + bias)
                s = st[r]
                off = 64 * s["k"]
                y_sb = wk.tile([128, 2, W], F16, tag="y", name="y_sb")
                u_sb = wk.tile([128, W], F32, tag="lr", name="u_sb")
                # xc-major so the first matmuls need only the first at-mult
                for xc in range(2):  # full 128-chunks of the x contraction
                    for uc in range(2):
                        nc.tensor.matmul(
                            s["py"][uc],
                            s["vut"][:, xc, 128 * uc : 128 * (uc + 1)],
                            s["at"][:, xc, :],
                            start=False,
                            stop=False,
                        )
                for uc in range(2):
                    py = s["py"][uc]
                    nc.tensor.matmul(  # 64-wide x-tail at partition offset
                        py,
                        s["vutt"][off : off + 64, 128 * uc : 128 * (uc + 1)],
                        s["at"][off : off + 64, 2, :],
                        start=False,
                        stop=True,
                    )
                    # LeakyReLU(t) = max(t, 0.2*t) with t = py + bias
                    bias_ap = bias_sb[:, uc : uc + 1]
                    nc.vector.tensor_scalar(
                        out=u_sb,
                        in0=py,
                        scalar1=bias_ap,
                        scalar2=0.2,
                        op0=mybir.AluOpType.add,
                        op1=mybir.AluOpType.mult,
                    )
                    nc.vector.scalar_tensor_tensor(
                        out=y_sb[:, uc, :],
                        in0=py,
                        scalar=bias_ap,
                        in1=u_sb,
                        op0=mybir.AluOpType.add,
                        op1=mybir.AluOpType.max,
                    )
                s["y"] = y_sb

            def z_out(r):
                # z = w2 @ y -> DRAM (fp16, host upcasts)
                s = st[r]
                z_sb = wk.tile([128, 2, W], F16, tag="z", name="z_sb")
                for oc in range(2):
                    pz = pp.tile([128, W], F32, tag="misc", bufs=2, name="pz")
                    for uc in range(2):
                        nc.tensor.matmul(
                            pz,
                            w2_sb[:, uc, 128 * oc : 128 * (oc + 1)],
                            s["y"][:, uc, :],
                            start=(uc == 0),
                            stop=(uc == 1),
                        )
                    nc.vector.tensor_copy(z_sb[:, oc, :], pz)
                nc.scalar.dma_start(out=out[r], in_=z_sb)
                # release per-row tiles
                del st[r]

            # ---- software-pipelined main loop ----
            # Per-iteration PE order: KG(r+1) first covers the previous row's
            # DVE epilogue before dots(r) reuses its PSUM ring slot; z(r-1)
            # sits between dots and the ones-sums so the last exp chunk is
            # ready; y_w1a+VUT (~5.1k cols) cover the 1/sum+scale DVE chain
            # before y_attn(r) consumes the normalized weights.
            kg_proj(0)
            vut_proj(0)
            for r in range(rows):
                if r % 2 == 0 and r // 2 + 2 < rows // 2:
                    dma_in(r // 2 + 2)
                if r + 1 < rows:
                    kg_proj(r + 1)
                dots_exp(r)
                if r >= 1:
                    z_out(r - 1)
                sums_norm(r)
                y_w1a(r)
                if r + 1 < rows:
                    vut_proj(r + 1)
                y_attn(r)
            z_out(rows - 1)
    _dedup_waits(nc)
    _cap_waits(nc)
    return nc


def fold_weights(wq, wk_, wv, w1, bn_gamma, bn_beta, bn_mean, bn_var, w2):
    """Host-side fp64 weight folding; returns the small device tensors."""
    f8 = np.float64
    s = bn_gamma.astype(f8) / np.sqrt(bn_var.astype(f8) + BN_EPS)
    w1s = w1.astype(f8) * s[:, None]
    w1a = w1s[:, :C]           # applies to left_feat
    w1b = w1s[:, C:]           # applies to the attention output
    U = w1b @ wv.astype(f8)    # [OUT, C]
    gTm = SCALE * (wk_.astype(f8).T @ wq.astype(f8))  # [c2, c1]
    bias = bn_beta.astype(f8) - bn_mean.astype(f8) * s
    return {
        "gT": np.ascontiguousarray(gTm, np.float16),
        "uT": np.ascontiguousarray(U.T, np.float16),
        "w1aT": np.ascontiguousarray(w1a.T, np.float16),
        "w2T": np.ascontiguousarray(w2.astype(f8).T, np.float16),
        "bnb": np.ascontiguousarray(bias, np.float32),
    }


def make_in_maps(inputs):
    left = np.asarray(inputs["left_feat"], np.float16)
    right = np.asarray(inputs["right_feat"], np.float16)
    common = fold_weights(
        np.asarray(inputs["wq"]),
        np.asarray(inputs["wk"]),
        np.asarray(inputs["wv"]),
        np.asarray(inputs["w1"]),
        np.asarray(inputs["bn_gamma"]),
        np.asarray(inputs["bn_beta"]),
        np.asarray(inputs["bn_mean"]),
        np.asarray(inputs["bn_var"]),
        np.asarray(inputs["w2"]),
    )
    in_maps = []
    for core in range(N_CORES):
        hs = slice(core * H_PER, (core + 1) * H_PER)
        # [ROWS, C, W] with row = (b, h)
        lf = left[:, :, hs, :].transpose(0, 2, 1, 3).reshape(ROWS, C, W)
        rf = right[:, :, hs, :].transpose(0, 2, 1, 3).reshape(ROWS, C, W)
        # pack to [ROWS, 128, 8, W]: chunks 0-3 = lf, 4-7 = rf; c = j*128 + p
        xin = np.concatenate(
            [
                lf.reshape(ROWS, 4, 128, W).transpose(0, 2, 1, 3),
                rf.reshape(ROWS, 4, 128, W).transpose(0, 2, 1, 3),
            ],
            axis=2,
        )
        in_maps.append({"xin": np.ascontiguousarray(xin), **common})
    return in_maps


def assemble_out(results):
    out = np.empty((B, OUT, H, W), np.float32)
    for core in range(N_CORES):
        # device layout [ROWS, 128, 2, W], channel o = oc*128 + p
        o = np.asarray(results[core]["out"]).astype(np.float32)
        o = o.transpose(0, 2, 1, 3).reshape(B, H_PER, OUT, W)
        out[:, :, core * H_PER : (core + 1) * H_PER, :] = o.transpose(0, 2, 1, 3)
    return out


_NC_CACHE: dict[int, bass.Bass] = {}


def get_nc(rows: int = ROWS) -> bass.Bass:
    if rows not in _NC_CACHE:
        _NC_CACHE[rows] = build_nc(rows)
    return _NC_CACHE[rows]


def run_sharded(inputs, **run_kwargs) -> bass_utils.BassKernelResults:
    """Run the SPMD kernel on all 8 cores; extra kwargs go to the runner
    (e.g. trace=True, trace_cores=[0] for NTFF profiling in test.py)."""
    in_maps = make_in_maps(inputs)
    nc = get_nc()
    return bass_utils.run_bass_kernel_spmd(
        nc, in_maps, core_ids=list(range(N_CORES)), **run_kwargs
    )


def kernel(**inputs) -> np.ndarray:
    return assemble_out(run_sharded(inputs).results)

